# revision 15
# baseline (speedup 1.0000x reference)
"""Routed-LoRA linear layer (moe_routing) on 8 trn2 NeuronCores.

Math (per token t):
  out[t, :] = W @ x[t] + b + 2.0 * sum_n mask[n, t] * (B_n @ (A_n @ x[t]))

Strategy (v2: fp8 DoubleRow main path):
  - Data-parallel over B*T = 65536 tokens: 8192 tokens per core.
  - The main matmul runs in fp8(e4m3) DoubleRow mode (K=256 per
    instruction, 0.5 PE cycles per output row) as a 3-term residual
    compensation at a single product scale of 2^6:
      t1: Q8(x) @ Q8(W*64)            [x_hi  @ W_hi6]
      t2: Q8((x-x_hi)*32) @ Q8(W*2)   [x_lo5 @ W_hi1]
      t3: Q8(x) @ Q8(W*64 - W_hi6)    [x_hi  @ W_lo6]
    All terms land at scale 2^6 in one fp32 PSUM accumulation group, so
    no device-side rescale is needed: the host divides the f32 output by
    64 (exact) and adds the bias in f32. Max-rel error ~9e-3 with dense
    masks (~3e-3 with one-hot), well inside the 2e-2 gate, at 25% less
    PE time than an all-bf16 main matmul would need -- and 4x less than
    the previous bf16 kernel's per-term cost.
  - LoRA: s.T = (A*64 fp8) @ x_hi computed directly in rank-partition
    layout [64, 512] per supertile (4 DoubleRow matmuls, no PE
    transpose), masked on DVE with a host-expanded per-(rank,token)
    bf16 mask (x SCALING), and accumulated into the base matmul's PSUM
    bank as a final bf16 K=64 contraction chunk.
  - Epilogue is a bare PSUM->SBUF bf16 copy (alternating DVE /
    Activation) + DMA; the 1/64 unscale and the bias ride on the host.
  - Preloads are spread across the scalar/gpsimd/vector DMA queues so
    supertile 0 can start as soon as the first (x,W) chunk pair lands;
    supertile 0 runs k-outer across six half-tiles to consume the
    preload stream at its delivery rate.
"""

import numpy as np
import ml_dtypes

import concourse.bass as bass
from concourse import bacc
import concourse.mybir as mybir
import concourse.tile as tile
from concourse.bass_utils import run_bass_kernel_spmd

N_CORES = 8
B, T = 8, 8192
D_IN = 1024
D_OUT = 1024
N_ADAPT, R = 4, 16
NR = N_ADAPT * R  # 64
SCALING = 32.0 / 16.0

TOK = B * T // N_CORES  # 8192 tokens per core
SUP = 512               # tokens per supertile
N_SUP = TOK // SUP      # 16
SUB = 128               # tokens per matmul M-tile
N_SUB = SUP // SUB      # 4
P = 128
KC = D_IN // P          # 8 contraction chunks of 128
NPAIR = KC // 2         # 4 DoubleRow chunk-pairs of 256
NB = D_OUT // 512       # 2 PSUM-bank column halves
S6 = 64.0               # product scale 2^6

F32 = mybir.dt.float32
BF16 = mybir.dt.bfloat16
F8 = mybir.dt.float8e4
NP_BF16 = ml_dtypes.bfloat16
NP_F8 = ml_dtypes.float8_e4m3
DR = mybir.MatmulPerfMode.DoubleRow


def build_bass():
    nc = bacc.Bacc(
        "TRN2", target_bir_lowering=False, debug=False, num_devices=N_CORES
    )

    xhi_d = nc.dram_tensor("xhi", [D_IN, TOK], F8, kind="ExternalInput")
    xlo_d = nc.dram_tensor("xlo", [D_IN, TOK], F8, kind="ExternalInput")
    w6_d = nc.dram_tensor("whi6", [D_IN, D_OUT], F8, kind="ExternalInput")
    w1_d = nc.dram_tensor("whi1", [D_IN, D_OUT], F8, kind="ExternalInput")
    wl_d = nc.dram_tensor("wlo6", [D_IN, D_OUT], F8, kind="ExternalInput")
    a8_d = nc.dram_tensor("a8", [P, KC * NR], F8, kind="ExternalInput")
    bt_d = nc.dram_tensor("btr", [NR, D_OUT], BF16, kind="ExternalInput")
    mj_d = nc.dram_tensor("mj", [NR, TOK], BF16, kind="ExternalInput")
    out_d = nc.dram_tensor("out", [TOK, D_OUT], BF16, kind="ExternalOutput")

    xhi_r = xhi_d.ap().rearrange("(kc p) t -> p kc t", p=P)
    xlo_r = xlo_d.ap().rearrange("(kc p) t -> p kc t", p=P)
    w6_r = w6_d.ap().rearrange("(kc p) n -> p kc n", p=P)
    w1_r = w1_d.ap().rearrange("(kc p) n -> p kc n", p=P)
    wl_r = wl_d.ap().rearrange("(kc p) n -> p kc n", p=P)
    out_r = out_d.ap().rearrange("(s q p) n -> s q p n", q=N_SUB, p=P)

    with tile.TileContext(nc) as tc:
        with (
            tc.tile_pool(name="const", bufs=1) as const,
            tc.tile_pool(name="xhp", bufs=3) as xhp,
            tc.tile_pool(name="xlp", bufs=3) as xlp,
            tc.tile_pool(name="smtp", bufs=2) as smtp,
            tc.tile_pool(name="op", bufs=6) as op,
            tc.tile_pool(name="pso", bufs=6, space="PSUM") as pso,
            tc.tile_pool(name="pst", bufs=2, space="PSUM") as pst,
        ):
            w6_sb = const.tile([P, KC, D_OUT], F8)
            w1_sb = const.tile([P, KC, D_OUT], F8)
            wl_sb = const.tile([P, KC, D_OUT], F8)
            a_sb = const.tile([P, KC, NR], F8)
            bt_sb = const.tile([NR, D_OUT], BF16)
            mj_sb = const.tile([NR, TOK], BF16)
            warm_sb = const.tile([P, 272], F8)

            # PE p-state warmup: the tensor engine ramps 0.65 -> 1.2 ->
            # 2.4 GHz over ~3us of continuous work, and the first real
            # matmul can't start until the first DMAs land (~3.8us).
            # Burn the ramp on zero matmuls so real work runs full-speed.
            nc.vector.memset(warm_sb[:], 0.0)
            warm_ps = pso.tile([P, 512], F32, tag="ops", name="warm")
            for i in range(12):
                nc.tensor.matmul(
                    warm_ps[:16, :256],
                    warm_sb[:, 0:16],
                    warm_sb[:, 16:272],
                    start=True,
                    stop=True,
                )

            # All DMA transfers serialize through one modeled DMA device
            # (and all descriptor gens through one HWDGE device), so the
            # preload ISSUE order is chosen to make the serial transfer
            # order match the s=0 consumption order: xh pair0, W_hi6
            # pair0, A, xh rest, W_hi6 rest, x_lo, mask0, W_lo6, W_hi1,
            # LoRA-B, mask rest. Supertile 0 consumes terms in t1 -> t3
            # -> t2 order to match.
            nc.scalar.dma_start(out=w6_sb[:, 0:2, :], in_=w6_r[:, 0:2, :])
            nc.scalar.dma_start(out=w6_sb[:, 2:4, :], in_=w6_r[:, 2:4, :])
            nc.scalar.dma_start(out=w6_sb[:, 4:8, :], in_=w6_r[:, 4:8, :])
            nc.scalar.dma_start(out=bt_sb[:], in_=bt_d.ap())
            nc.gpsimd.dma_start(
                out=a_sb[:],
                in_=a8_d.ap().rearrange("p (kc j) -> p kc j", kc=KC),
            )
            nc.gpsimd.dma_start(out=mj_sb[:, :SUP], in_=mj_d.ap()[:, :SUP])
            nc.gpsimd.dma_start(out=wl_sb[:], in_=wl_r[:])
            nc.gpsimd.dma_start(out=w1_sb[:], in_=w1_r[:])
            nc.gpsimd.dma_start(
                out=mj_sb[:, SUP : 8 * SUP], in_=mj_d.ap()[:, SUP : 8 * SUP]
            )
            nc.gpsimd.dma_start(
                out=mj_sb[:, 8 * SUP :], in_=mj_d.ap()[:, 8 * SUP :]
            )

            def mm(ops_t, x_sb, w_sb, c, ts, nsl, start=False, stop=False):
                nc.tensor.matmul(
                    ops_t[:],
                    x_sb[:, 2 * c : 2 * c + 2, ts : ts + SUB],
                    w_sb[:, 2 * c : 2 * c + 2, nsl],
                    start=start,
                    stop=stop,
                    perf_mode=DR,
                )

            for s in range(N_SUP):
                t0 = s * SUP
                xh = xhp.tile([P, KC, SUP], F8, tag="xh")
                xl = xlp.tile([P, KC, SUP], F8, tag="xl")
                if s == 0:
                    nc.sync.dma_start(
                        out=xh[:, 0:2, :], in_=xhi_r[:, 0:2, t0 : t0 + SUP]
                    )
                    nc.sync.dma_start(
                        out=xh[:, 2:8, :], in_=xhi_r[:, 2:8, t0 : t0 + SUP]
                    )
                else:
                    nc.sync.dma_start(out=xh[:], in_=xhi_r[:, :, t0 : t0 + SUP])
                nc.sync.dma_start(out=xl[:], in_=xlo_r[:, :, t0 : t0 + SUP])

                ps_t = pst.tile([NR, SUP], F32, tag="pst", name=f"pst{s}")
                smt = smtp.tile([NR, SUP], BF16, tag="smt", name=f"smt{s}")

                def sT():
                    # s.T = (A*64).T-stationary @ x_hi: rank-partition
                    # layout directly, no PE transpose needed
                    for c in range(NPAIR):
                        nc.tensor.matmul(
                            ps_t[:],
                            a_sb[:, 2 * c : 2 * c + 2, :],
                            xh[:, 2 * c : 2 * c + 2, :],
                            start=(c == 0),
                            stop=(c == NPAIR - 1),
                            perf_mode=DR,
                        )

                def mask_mul():
                    nc.vector.tensor_mul(
                        smt[:], ps_t[:], mj_sb[:, t0 : t0 + SUP]
                    )

                def lora_store(q, n, ops_t):
                    ts = q * SUB
                    nsl = slice(n * 512, (n + 1) * 512)
                    nc.tensor.matmul(
                        ops_t[:],
                        smt[:, ts : ts + SUB],
                        bt_sb[:, nsl],
                        start=False,
                        stop=True,
                    )
                    # PSUM can't be DMA'd directly; stage through SBUF.
                    # Alternate the copy between DVE and Activation so
                    # neither engine exceeds ~25% busy.
                    o_sb = op.tile([P, 512], BF16, tag="o")
                    if n == 0:
                        nc.vector.tensor_copy(o_sb[:], ops_t[:])
                    else:
                        nc.scalar.activation(
                            o_sb[:], ops_t[:], mybir.ActivationFunctionType.Copy
                        )
                    nc.scalar.dma_start(out=out_r[s, q][:, nsl], in_=o_sb[:])

                if s == 0:
                    # k-outer across six half-tiles in term-waves, each
                    # wave gated on one big preload DMA: t1 wave A (W_hi6
                    # first half), t1 wave B (second half), t2 (W_hi1),
                    # sT (A), t3 (W_lo6), so PE never waits long.
                    ph = {}
                    for q in range(3):
                        for n in range(NB):
                            ph[q, n] = pso.tile(
                                [P, 512], F32, tag="ops", name=f"ops0_{q}_{n}"
                            )
                    for q in range(3):
                        for n in range(NB):
                            mm(ph[q, n], xh, w6_sb, 0, q * SUB,
                               slice(n * 512, (n + 1) * 512), start=True)
                    sT()
                    mask_mul()
                    for c in (1, 2, 3):
                        for q in range(3):
                            for n in range(NB):
                                mm(ph[q, n], xh, w6_sb, c, q * SUB,
                                   slice(n * 512, (n + 1) * 512))
                    for c in range(NPAIR):
                        for q in range(3):
                            for n in range(NB):
                                mm(ph[q, n], xh, wl_sb, c, q * SUB,
                                   slice(n * 512, (n + 1) * 512))
                    for c in range(NPAIR):
                        for q in range(3):
                            for n in range(NB):
                                mm(ph[q, n], xl, w1_sb, c, q * SUB,
                                   slice(n * 512, (n + 1) * 512))
                    for q in range(3):
                        for n in range(NB):
                            lora_store(q, n, ph[q, n])
                    q_range = (3,)
                else:
                    sT()
                    mask_mul()
                    q_range = range(N_SUB)

                for q in q_range:
                    ts = q * SUB
                    last = s == N_SUP - 1 and q == N_SUB - 1

                    def mains(n):
                        t = pso.tile(
                            [P, 512], F32, tag="ops", name=f"ops{s}_{q}_{n}"
                        )
                        nsl = slice(n * 512, (n + 1) * 512)
                        for c in range(NPAIR):
                            mm(t, xh, w6_sb, c, ts, nsl, start=(c == 0))
                        for c in range(NPAIR):
                            mm(t, xl, w1_sb, c, ts, nsl)
                        for c in range(NPAIR):
                            mm(t, xh, wl_sb, c, ts, nsl)
                        return t

                    if not last:
                        ops = {n: mains(n) for n in range(NB)}
                        for n in range(NB):
                            lora_store(q, n, ops[n])
                    else:
                        # Final tile: drain half 1 while half 0's mains
                        # run so only one store remains in the tail.
                        ops1 = mains(1)
                        lora_store(q, 1, ops1)
                        ops0 = mains(0)
                        lora_store(q, 0, ops0)

    nc.compile()
    return nc


_NC_CACHE = None


def _get_nc():
    global _NC_CACHE
    if _NC_CACHE is None:
        _NC_CACHE = build_bass()
    return _NC_CACHE


def make_in_maps(x, W, b, lora_A, lora_B, masks):
    x = np.ascontiguousarray(x, dtype=np.float32)
    W = np.ascontiguousarray(W, dtype=np.float32)
    lora_A = np.ascontiguousarray(lora_A, dtype=np.float32)
    lora_B = np.ascontiguousarray(lora_B, dtype=np.float32)
    masks = np.ascontiguousarray(masks, dtype=np.float32)

    x_flat = x.reshape(B * T, D_IN)
    A_flat = lora_A.reshape(NR, D_IN)
    B_flat = lora_B.transpose(1, 0, 2).reshape(D_OUT, NR)

    # fp8 residual split of x (shared across cores, then sliced)
    x_hi8 = x_flat.astype(NP_F8)
    x_hi32 = x_hi8.astype(np.float32)
    x_lo8 = ((x_flat - x_hi32) * 32.0).astype(NP_F8)

    Wt = np.ascontiguousarray(W.T)                    # [D_IN, D_OUT]
    w_hi6 = (Wt * S6).astype(NP_F8)
    w_hi1 = (Wt * 2.0).astype(NP_F8)
    w_lo6 = (Wt * S6 - w_hi6.astype(np.float32)).astype(NP_F8)

    a8_full = (A_flat * S6).astype(NP_F8)             # [NR, D_IN]
    a8 = np.ascontiguousarray(
        a8_full.T.reshape(KC, P, NR).transpose(1, 0, 2).reshape(P, KC * NR)
    )
    btr = np.ascontiguousarray(B_flat.T.astype(NP_BF16))  # [NR, D_OUT]

    # per-(rank, token) mask with the LoRA scaling folded in
    m_full = masks[..., 0].reshape(N_ADAPT, B * T) * np.float32(SCALING)
    mj_full = np.repeat(m_full, R, axis=0)            # [NR, B*T]

    in_maps = []
    for c in range(N_CORES):
        sl = slice(c * TOK, (c + 1) * TOK)
        in_maps.append(
            {
                "xhi": np.ascontiguousarray(x_hi8[sl].T),
                "xlo": np.ascontiguousarray(x_lo8[sl].T),
                "whi6": w_hi6,
                "whi1": w_hi1,
                "wlo6": w_lo6,
                "a8": a8,
                "btr": btr,
                "mj": np.ascontiguousarray(mj_full[:, sl].astype(NP_BF16)),
            }
        )
    return in_maps


def kernel(x, W, b, lora_A, lora_B, masks):
    nc = _get_nc()
    in_maps = make_in_maps(x, W, b, lora_A, lora_B, masks)
    res = run_bass_kernel_spmd(nc, in_maps, core_ids=list(range(N_CORES)))
    out = np.concatenate([r["out"] for r in res.results], axis=0)
    out = out.astype(np.float32) * np.float32(1.0 / S6)
    out += np.asarray(b, dtype=np.float32)[None, :]
    return out.reshape(B, T, D_OUT)


# revision 16
# speedup vs baseline: 1.0719x; 1.0719x over previous
"""Routed-LoRA linear layer (moe_routing) on 8 trn2 NeuronCores.

Math (per token t):
  out[t, :] = W @ x[t] + b + 2.0 * sum_n mask[n, t] * (B_n @ (A_n @ x[t]))

Strategy (v3: fp8 DoubleRow everywhere):
  - Data-parallel over B*T = 65536 tokens: 8192 tokens per core.
  - The main matmul runs in fp8(e4m3) DoubleRow mode (K=256 per
    instruction, 0.5 PE cycles per output row) as a 3-term residual
    compensation at a single product scale of 2^6:
      t1: Q8(x)        @ Q8(W*64)             [x_hi @ W_hi6]
      t2: Q8(x - x_hi) @ Q8(W*64)             [x_lo @ W_hi6]
      t3: Q8(x)        @ Q8(W*64 - W_hi6)     [x_hi @ W_lo6]
    t2 reuses W_hi6 (x_lo kept at scale 2^0), so only two W images are
    preloaded. All terms land in one fp32 PSUM group; the host divides
    the bf16 output by 64 (exact) and adds the bias in f32.
  - LoRA: s.T = (A*64 fp8) @ x_hi computed directly in rank-partition
    layout [64, 512] per supertile (4 DoubleRow matmuls, no PE
    transpose), masked on DVE into fp8 at scale 2^1, DoubleRow-packed
    [32, 2, tok] via a partition-shifting SBUF->SBUF DMA, and
    accumulated into the base matmul's PSUM bank as a final fp8
    DoubleRow chunk against (B*32 fp8). Max-rel error ~1.2e-2, inside
    the 2e-2 gate.
  - Epilogue: bare PSUM->SBUF bf16 copy (DVE for half 0, Activation for
    half 1) + DMA; the 1/64 unscale and the bias ride on the host.
  - The cost model serializes all DMA transfers through one device and
    all HWDGE descriptor gens through another, so preloads stream in
    consumption order on the scalar queue; supertile 0 consumes terms
    k-outer in arrival order (t1 by chunk-pair, t2, t3) with zero-data
    PE "warmup" matmuls bridging the gaps so the tensor engine's
    p-state ramp (0.65 -> 1.2 -> 2.4 GHz over ~3us of *continuous*
    work) is never reset by an idle gap.
"""

import numpy as np
import ml_dtypes

import concourse.bass as bass
from concourse import bacc
import concourse.mybir as mybir
import concourse.tile as tile
from concourse.bass_utils import run_bass_kernel_spmd

N_CORES = 8
B, T = 8, 8192
D_IN = 1024
D_OUT = 1024
N_ADAPT, R = 4, 16
NR = N_ADAPT * R  # 64
SCALING = 32.0 / 16.0

TOK = B * T // N_CORES  # 8192 tokens per core
SUP = 512               # tokens per supertile
N_SUP = TOK // SUP      # 16
SUB = 128               # tokens per matmul M-tile
N_SUB = SUP // SUB      # 4
P = 128
KC = D_IN // P          # 8 contraction chunks of 128
NPAIR = KC // 2         # 4 DoubleRow chunk-pairs of 256
NB = D_OUT // 512       # 2 PSUM-bank column halves
S6 = 64.0               # product scale 2^6

F32 = mybir.dt.float32
BF16 = mybir.dt.bfloat16
F8 = mybir.dt.float8e4
NP_BF16 = ml_dtypes.bfloat16
NP_F8 = ml_dtypes.float8_e4m3
DR = mybir.MatmulPerfMode.DoubleRow

# warmup-bridge sizes for supertile 0 (tuned against the trace)
WARM0 = 12       # before any real work (PE start ~1.2us, data ~3.9us)
BR_T1C0 = 5      # t1 c0 done -> xh-rest/a8 (sT)
BR_ST = 2        # sT done -> w6 pair 1
BR_T1C1 = 8      # t1 c1 done -> x_lo (t2 c0)
BR_T2C0 = 1      # t2 c0 done -> w6 pair 2
BR_T2C1 = 2      # t2 c1 done -> wl pair 0


def build_bass():
    nc = bacc.Bacc(
        "TRN2", target_bir_lowering=False, debug=False, num_devices=N_CORES
    )

    xhi_d = nc.dram_tensor("xhi", [D_IN, TOK], F8, kind="ExternalInput")
    xlo_d = nc.dram_tensor("xlo", [D_IN, TOK], F8, kind="ExternalInput")
    w6_d = nc.dram_tensor("whi6", [D_IN, D_OUT], F8, kind="ExternalInput")
    wl_d = nc.dram_tensor("wlo6", [D_IN, D_OUT], F8, kind="ExternalInput")
    a8_d = nc.dram_tensor("a8", [P, KC * NR], F8, kind="ExternalInput")
    bt8_d = nc.dram_tensor("bt8", [NR // 2, 2 * D_OUT], F8, kind="ExternalInput")
    mj_d = nc.dram_tensor("mj", [NR, TOK], BF16, kind="ExternalInput")
    out_d = nc.dram_tensor("out", [TOK, D_OUT], BF16, kind="ExternalOutput")

    xhi_r = xhi_d.ap().rearrange("(kc p) t -> p kc t", p=P)
    xlo_r = xlo_d.ap().rearrange("(kc p) t -> p kc t", p=P)
    w6_r = w6_d.ap().rearrange("(kc p) n -> p kc n", p=P)
    wl_r = wl_d.ap().rearrange("(kc p) n -> p kc n", p=P)
    out_r = out_d.ap().rearrange("(s q p) n -> s q p n", q=N_SUB, p=P)

    with tile.TileContext(nc) as tc:
        with (
            tc.tile_pool(name="const", bufs=1) as const,
            tc.tile_pool(name="xhp", bufs=3) as xhp,
            tc.tile_pool(name="xlp", bufs=3) as xlp,
            tc.tile_pool(name="smtp", bufs=2) as smtp,
            tc.tile_pool(name="smhp", bufs=2) as smhp,
            tc.tile_pool(name="op", bufs=6) as op,
            tc.tile_pool(name="pso", bufs=7, space="PSUM") as pso,
            tc.tile_pool(name="warmp", bufs=1, space="PSUM") as warmp,
        ):
            w6_sb = const.tile([P, KC, D_OUT], F8)
            wl_sb = const.tile([P, KC, D_OUT], F8)
            a_sb = const.tile([P, KC, NR], F8)
            bt_sb = const.tile([NR // 2, 2, D_OUT], F8)
            mj_sb = const.tile([NR, TOK], BF16)
            warm_sb = const.tile([P, 272], F8)

            warm_ps = warmp.tile([P, 512], F32)
            nc.vector.memset(warm_sb[:], 0.0)

            def bridge(k):
                for _ in range(k):
                    nc.tensor.matmul(
                        warm_ps[:16, :256],
                        warm_sb[:, 0:16],
                        warm_sb[:, 16:272],
                        start=True,
                        stop=True,
                    )

            bridge(WARM0)

            # preloads: scalar queue in exact consumption order (the
            # sync queue's x loads interleave into the serial transfer
            # stream between these)
            nc.scalar.dma_start(out=w6_sb[:, 0:2, :], in_=w6_r[:, 0:2, :])
            nc.scalar.dma_start(
                out=a_sb[:],
                in_=a8_d.ap().rearrange("p (kc j) -> p kc j", kc=KC),
            )
            nc.scalar.dma_start(out=w6_sb[:, 2:4, :], in_=w6_r[:, 2:4, :])
            nc.scalar.dma_start(out=w6_sb[:, 4:6, :], in_=w6_r[:, 4:6, :])
            nc.scalar.dma_start(out=w6_sb[:, 6:8, :], in_=w6_r[:, 6:8, :])
            nc.scalar.dma_start(
                out=bt_sb[:],
                in_=bt8_d.ap().rearrange("p (i n) -> p i n", i=2),
            )
            for c in range(NPAIR):
                nc.scalar.dma_start(
                    out=wl_sb[:, 2 * c : 2 * c + 2, :],
                    in_=wl_r[:, 2 * c : 2 * c + 2, :],
                )
            nc.scalar.dma_start(
                out=mj_sb[:, SUP : 8 * SUP], in_=mj_d.ap()[:, SUP : 8 * SUP]
            )
            nc.scalar.dma_start(
                out=mj_sb[:, 8 * SUP :], in_=mj_d.ap()[:, 8 * SUP :]
            )
            # gpsimd (SWDGE): tiny first mask slice, lands early
            nc.gpsimd.dma_start(out=mj_sb[:, :SUP], in_=mj_d.ap()[:, :SUP])

            def mm(ops_t, x_sb, w_sb, c, ts, nsl, start=False, stop=False):
                nc.tensor.matmul(
                    ops_t[:],
                    x_sb[:, 2 * c : 2 * c + 2, ts : ts + SUB],
                    w_sb[:, 2 * c : 2 * c + 2, nsl],
                    start=start,
                    stop=stop,
                    perf_mode=DR,
                )

            x_tiles = {}  # pre-issued x tiles (supertile 1)

            for s in range(N_SUP):
                t0 = s * SUP
                if s in x_tiles:
                    xh, xl = x_tiles.pop(s)
                else:
                    xh = xhp.tile([P, KC, SUP], F8, tag="xh")
                    xl = xlp.tile([P, KC, SUP], F8, tag="xl")
                    if s == 0:
                        nc.sync.dma_start(
                            out=xh[:, 0:2, :], in_=xhi_r[:, 0:2, t0 : t0 + SUP]
                        )
                        nc.sync.dma_start(
                            out=xh[:, 2:8, :], in_=xhi_r[:, 2:8, t0 : t0 + SUP]
                        )
                    else:
                        nc.sync.dma_start(
                            out=xh[:], in_=xhi_r[:, :, t0 : t0 + SUP]
                        )
                    nc.sync.dma_start(out=xl[:], in_=xlo_r[:, :, t0 : t0 + SUP])

                ps_t = pso.tile([P, 512], F32, tag="ops", name=f"pst{s}")
                smt = smtp.tile([NR // 2, 2, SUP], F8, tag="smt", name=f"smt{s}")
                smh = smhp.tile([NR, SUP], F8, tag="smh", name=f"smh{s}")

                def sT():
                    for c in range(NPAIR):
                        nc.tensor.matmul(
                            ps_t[:NR, :],
                            a_sb[:, 2 * c : 2 * c + 2, :],
                            xh[:, 2 * c : 2 * c + 2, :],
                            start=(c == 0),
                            stop=(c == NPAIR - 1),
                            perf_mode=DR,
                        )

                def mask_pack():
                    # mask+quantize to fp8 in DoubleRow-packed layout:
                    # ranks 0..31 written in place, ranks 32..63 shifted
                    # down 32 partitions by a SBUF->SBUF DMA
                    nc.vector.tensor_mul(
                        smt[:, 0, :], ps_t[0:32, :], mj_sb[0:32, t0 : t0 + SUP]
                    )
                    nc.vector.tensor_mul(
                        smh[32:64, :], ps_t[32:64, :],
                        mj_sb[32:64, t0 : t0 + SUP],
                    )
                    nc.sync.dma_start(out=smt[:, 1, :], in_=smh[32:64, :])

                def lora_store(q, n, ops_t):
                    ts = q * SUB
                    nsl = slice(n * 512, (n + 1) * 512)
                    nc.tensor.matmul(
                        ops_t[:],
                        smt[:, :, ts : ts + SUB],
                        bt_sb[:, :, nsl],
                        start=False,
                        stop=True,
                        perf_mode=DR,
                    )
                    o_sb = op.tile([P, 512], BF16, tag="o")
                    if n == 0:
                        nc.vector.tensor_copy(o_sb[:], ops_t[:])
                    else:
                        nc.scalar.activation(
                            o_sb[:], ops_t[:], mybir.ActivationFunctionType.Copy
                        )
                    nc.scalar.dma_start(out=out_r[s, q][:, nsl], in_=o_sb[:])

                def mains(q, n, name):
                    t = pso.tile([P, 512], F32, tag="ops", name=name)
                    ts = q * SUB
                    nsl = slice(n * 512, (n + 1) * 512)
                    for c in range(NPAIR):
                        mm(t, xh, w6_sb, c, ts, nsl, start=(c == 0))
                    for c in range(NPAIR):
                        mm(t, xl, w6_sb, c, ts, nsl)
                    for c in range(NPAIR):
                        mm(t, xh, wl_sb, c, ts, nsl)
                    return t

                if s == 0:
                    # k-outer across six half-tiles, term waves in DMA
                    # arrival order, warmup bridges over the data gaps
                    ph = {}
                    for q in range(3):
                        for n in range(NB):
                            ph[q, n] = pso.tile(
                                [P, 512], F32, tag="ops", name=f"ops0_{q}_{n}"
                            )

                    def wave(x_sb, w_sb, c, start=False):
                        for q in range(3):
                            for n in range(NB):
                                mm(ph[q, n], x_sb, w_sb, c, q * SUB,
                                   slice(n * 512, (n + 1) * 512), start=start)

                    wave(xh, w6_sb, 0, start=True)   # t1 c0
                    bridge(BR_T1C0)
                    sT()
                    mask_pack()
                    bridge(BR_ST)
                    wave(xh, w6_sb, 1)               # t1 c1
                    bridge(BR_T1C1)
                    # release supertile 1's x loads now that the mask is
                    # done: their DMA descriptors would otherwise jump
                    # ahead of the critical preloads in the serial queue
                    xh1 = xhp.tile([P, KC, SUP], F8, tag="xh")
                    xl1 = xlp.tile([P, KC, SUP], F8, tag="xl")
                    nc.vector.memset(xh1[0:1, 0:1, 0:1], 0.0)
                    nc.vector.memset(xl1[0:1, 0:1, 0:1], 0.0)
                    nc.sync.dma_start(out=xh1[:], in_=xhi_r[:, :, SUP : 2 * SUP])
                    nc.sync.dma_start(out=xl1[:], in_=xlo_r[:, :, SUP : 2 * SUP])
                    x_tiles[1] = (xh1, xl1)
                    wave(xl, w6_sb, 0)               # t2 c0
                    bridge(BR_T2C0)
                    wave(xh, w6_sb, 2)               # t1 c2
                    wave(xh, w6_sb, 3)               # t1 c3
                    wave(xl, w6_sb, 1)               # t2 c1
                    bridge(BR_T2C1)
                    wave(xh, wl_sb, 0)               # t3 c0
                    wave(xl, w6_sb, 2)               # t2 c2
                    wave(xh, wl_sb, 1)               # t3 c1
                    wave(xl, w6_sb, 3)               # t2 c3
                    wave(xh, wl_sb, 2)               # t3 c2
                    wave(xh, wl_sb, 3)               # t3 c3
                    # free ph[0,*] banks early so q3's tiles can start
                    for n in range(NB):
                        lora_store(0, n, ph[0, n])
                    ops30 = mains(3, 0, "ops0_3_0")
                    ops31 = mains(3, 1, "ops0_3_1")
                    for q in (1, 2):
                        for n in range(NB):
                            lora_store(q, n, ph[q, n])
                    lora_store(3, 0, ops30)
                    lora_store(3, 1, ops31)
                else:
                    # steady state: loraB(q) staggered one tile behind
                    # mains(q) so the mask/pack round trip is hidden
                    sT()
                    mask_pack()
                    ops = {}
                    last = s == N_SUP - 1
                    ops[0, 0] = mains(0, 0, f"ops{s}_0_0")
                    ops[0, 1] = mains(0, 1, f"ops{s}_0_1")
                    ops[1, 0] = mains(1, 0, f"ops{s}_1_0")
                    ops[1, 1] = mains(1, 1, f"ops{s}_1_1")
                    lora_store(0, 0, ops[0, 0])
                    lora_store(0, 1, ops[0, 1])
                    ops[2, 0] = mains(2, 0, f"ops{s}_2_0")
                    ops[2, 1] = mains(2, 1, f"ops{s}_2_1")
                    lora_store(1, 0, ops[1, 0])
                    lora_store(1, 1, ops[1, 1])
                    ops[3, 0] = mains(3, 0, f"ops{s}_3_0")
                    ops[3, 1] = mains(3, 1, f"ops{s}_3_1")
                    lora_store(2, 0, ops[2, 0])
                    lora_store(2, 1, ops[2, 1])
                    if last:
                        # half 1 first so only one store drains in the tail
                        lora_store(3, 1, ops[3, 1])
                        lora_store(3, 0, ops[3, 0])
                    else:
                        lora_store(3, 0, ops[3, 0])
                        lora_store(3, 1, ops[3, 1])

    nc.compile()
    return nc


_NC_CACHE = None


def _get_nc():
    global _NC_CACHE
    if _NC_CACHE is None:
        _NC_CACHE = build_bass()
    return _NC_CACHE


def make_in_maps(x, W, b, lora_A, lora_B, masks):
    x = np.ascontiguousarray(x, dtype=np.float32)
    W = np.ascontiguousarray(W, dtype=np.float32)
    lora_A = np.ascontiguousarray(lora_A, dtype=np.float32)
    lora_B = np.ascontiguousarray(lora_B, dtype=np.float32)
    masks = np.ascontiguousarray(masks, dtype=np.float32)

    x_flat = x.reshape(B * T, D_IN)
    A_flat = lora_A.reshape(NR, D_IN)
    B_flat = lora_B.transpose(1, 0, 2).reshape(D_OUT, NR)

    x_hi8 = x_flat.astype(NP_F8)
    x_hi32 = x_hi8.astype(np.float32)
    x_lo8 = (x_flat - x_hi32).astype(NP_F8)

    Wt = np.ascontiguousarray(W.T)                    # [D_IN, D_OUT]
    w_hi6 = (Wt * S6).astype(NP_F8)
    w_lo6 = (Wt * S6 - w_hi6.astype(np.float32)).astype(NP_F8)

    a8_full = (A_flat * S6).astype(NP_F8)             # [NR, D_IN]
    a8 = np.ascontiguousarray(
        a8_full.T.reshape(KC, P, NR).transpose(1, 0, 2).reshape(P, KC * NR)
    )
    # B rows at scale 2^5, DoubleRow-packed: row (p, i) holds j = i*32+p
    b8 = (B_flat.T * 32.0).astype(NP_F8)              # [NR, D_OUT]
    bt8 = np.ascontiguousarray(
        b8.reshape(2, NR // 2, D_OUT).transpose(1, 0, 2).reshape(NR // 2, -1)
    )

    # per-(rank, token) mask: smt = (s*64) * mj = s_masked * 2^1
    m_full = masks[..., 0].reshape(N_ADAPT, B * T) * np.float32(
        SCALING * 2.0 / S6
    )
    mj_full = np.repeat(m_full, R, axis=0)            # [NR, B*T]

    in_maps = []
    for c in range(N_CORES):
        sl = slice(c * TOK, (c + 1) * TOK)
        in_maps.append(
            {
                "xhi": np.ascontiguousarray(x_hi8[sl].T),
                "xlo": np.ascontiguousarray(x_lo8[sl].T),
                "whi6": w_hi6,
                "wlo6": w_lo6,
                "a8": a8,
                "bt8": bt8,
                "mj": np.ascontiguousarray(mj_full[:, sl].astype(NP_BF16)),
            }
        )
    return in_maps


def kernel(x, W, b, lora_A, lora_B, masks):
    nc = _get_nc()
    in_maps = make_in_maps(x, W, b, lora_A, lora_B, masks)
    res = run_bass_kernel_spmd(nc, in_maps, core_ids=list(range(N_CORES)))
    out = np.concatenate([r["out"] for r in res.results], axis=0)
    out = out.astype(np.float32) * np.float32(1.0 / S6)
    out += np.asarray(b, dtype=np.float32)[None, :]
    return out.reshape(B, T, D_OUT)
